# revision 15
# baseline (speedup 1.0000x reference)
"""Multi-head causal+padded attention on 8 Trainium2 NeuronCores.

Sharding: core c handles batch b = c//2 and head-group g = c%2 (8 of 16 heads).
Each core computes its q/k/v projections (512 output dims) and attention for
its 8 heads over the full 2048-seq, producing out^T [512, 2048]; the host
transposes/concats into the full [4, 2048, 1024] output.

Device algorithm (per core), v2:
  All matmul operands bf16 (fp32 PSUM accumulation). xT resident in SBUF;
  qT/kT = W^T-slices @ xT ([outdim, seq] layout); v natural [seq, outdim],
  augmented with a 65th all-ones column whose att-weighted sum is the softmax
  denominator. Key-side pad masking is folded into the exp bias (-87 for
  padded keys). Scores for a head pair are computed as two row-tiled matmuls
  (partitions 0-63 / 64-127 of the contraction) into adjacent PSUM banks and
  exp'd by a single paired ACT instruction. AV matmuls run one k-block behind
  QK/exp (software pipeline). Projections for seq-chunk scn+1 are interleaved
  into the attention of chunk scn as micro-ops to keep the PE busy while the
  ACT engine drains exp work. Normalization: copy PSUM->SBUF, fast-approx
  reciprocal of the denominator row, pad(q) fold, gpsimd partition broadcast,
  one elementwise multiply.
"""
import os
import sys

sys.path.insert(0, "/opt/trn_rl_repo")

import numpy as np
import ml_dtypes

S = 2048
E = 1024
D = 64
H = 16          # total heads
HPC = 8         # heads per core
OC = HPC * D    # 512 output dims per core
EB = E // 128   # 8 contraction blocks
NSB = S // 128  # 16 seq blocks
NCH = S // 512  # 4 q-chunks
B = 4
NCORES = 8

SCH_A = 184.6650172  # 128/ln(2)

_cache = {}


def _build_nc():
    from concourse import bacc
    import concourse.tile as tile
    import concourse.mybir as mybir

    F32 = mybir.dt.float32
    BF16 = mybir.dt.bfloat16
    AF = mybir.ActivationFunctionType
    ALU = mybir.AluOpType

    nc = bacc.Bacc("TRN2", target_bir_lowering=False, debug=False,
                   num_devices=NCORES)
    xT = nc.dram_tensor("xT", [E, S], BF16, kind="ExternalInput").ap()
    wqT = nc.dram_tensor("wqT", [E, OC], BF16, kind="ExternalInput").ap()
    wkT = nc.dram_tensor("wkT", [E, OC], BF16, kind="ExternalInput").ap()
    wvT = nc.dram_tensor("wvT", [E, OC], BF16, kind="ExternalInput").ap()
    bq = nc.dram_tensor("bq", [OC], F32, kind="ExternalInput").ap()
    bk = nc.dram_tensor("bk", [OC], F32, kind="ExternalInput").ap()
    bv = nc.dram_tensor("bv", [OC], F32, kind="ExternalInput").ap()
    pad = nc.dram_tensor("pad", [S], F32, kind="ExternalInput").ap()
    outT = nc.dram_tensor("outT", [OC, S], F32, kind="ExternalOutput").ap()

    with tile.TileContext(nc) as tc:
        with tc.tile_pool(name="const", bufs=1) as cpool, \
             tc.tile_pool(name="big", bufs=1) as bigpool:

            # ---------------- big SBUF residents ----------------
            x_sb = bigpool.tile([128, EB * S], BF16, tag="x_sb")
            wq_sb = bigpool.tile([128, EB * OC], BF16, tag="wq_sb")
            wk_sb = bigpool.tile([128, EB * OC], BF16, tag="wk_sb")
            wv_sb = bigpool.tile([128, EB * OC], BF16, tag="wv_sb")
            qT_sb = bigpool.tile([128, 4 * S], BF16, tag="qT")
            kT_sb = bigpool.tile([128, 4 * S], BF16, tag="kT")
            v_aug = bigpool.tile([128, NSB * HPC * 65], BF16, tag="v_aug")
            v_r = v_aug[:].rearrange("p (b h c) -> p b h c", b=NSB, h=HPC)

            # DMA order = critical path of attn(0, hp0): all x blocks, the
            # ob0 column-slices of wq/wk, wv, then the weight remainders.
            for eb in range(EB):
                nc.sync.dma_start(x_sb[:, eb * S:(eb + 1) * S],
                                  xT[eb * 128:(eb + 1) * 128, :])
            wq_v = wq_sb[:].rearrange("p (e c) -> p e c", e=EB)
            wk_v = wk_sb[:].rearrange("p (e c) -> p e c", e=EB)
            wqT_v = wqT.rearrange("(e p) c -> p e c", p=128)
            wkT_v = wkT.rearrange("(e p) c -> p e c", p=128)
            nc.sync.dma_start(wq_v[:, :, 0:128], wqT_v[:, :, 0:128])
            nc.sync.dma_start(wk_v[:, :, 0:128], wkT_v[:, :, 0:128])
            nc.sync.dma_start(
                wv_sb[:].rearrange("p (e c) -> p e c", e=EB),
                wvT.rearrange("(e p) c -> p e c", p=128))
            nc.sync.dma_start(wq_v[:, :, 128:OC], wqT_v[:, :, 128:OC])
            nc.sync.dma_start(wk_v[:, :, 128:OC], wkT_v[:, :, 128:OC])

            # ---------------- constants ----------------
            pad_sb = cpool.tile([128, NSB], F32, tag="pad_f")
            nc.sync.dma_start(pad_sb[:], pad.rearrange("(b p) -> p b", p=128))
            pad_row = cpool.tile([1, S], F32, tag="padr_f")
            nc.sync.dma_start(pad_row[:], pad.rearrange("(a s) -> a s", a=1))
            # exp bias: 0 where pad=1, -87 where pad=0 (folds key padding)
            padlog = cpool.tile([128, NSB], F32, tag="padlog")
            nc.vector.tensor_scalar(padlog[:], pad_sb[:], 87.0, -87.0,
                                    ALU.mult, ALU.add)
            # Schraudolph bias for the DVE exp path (C = -7)
            sch_bias = cpool.tile([128, NSB], F32, tag="sch_bias")
            nc.vector.tensor_scalar(sch_bias[:], padlog[:], SCH_A, 16249.0,
                                    ALU.mult, ALU.add)

            bq_sb = cpool.tile([128, 4], F32, tag="bq")
            nc.sync.dma_start(bq_sb[:], bq.rearrange("(b p) -> p b", p=128))
            bk_sb = cpool.tile([128, 4], F32, tag="bk")
            nc.sync.dma_start(bk_sb[:], bk.rearrange("(b p) -> p b", p=128))
            bv_row = cpool.tile([1, OC], F32, tag="bv_row")
            nc.sync.dma_start(bv_row[:], bv.rearrange("(a c) -> a c", a=1))
            bv_tile = cpool.tile([128, OC], F32, tag="bv_tile")
            nc.gpsimd.partition_broadcast(bv_tile[:], bv_row[:])

            # tri[k, q] = 1 where k <= q else 0 (local 128x128 diagonal block)
            tri = cpool.tile([128, 128], BF16, tag="tri")
            nc.gpsimd.memset(tri[:], 1.0)
            nc.gpsimd.affine_select(
                out=tri[:], in_=tri[:], compare_op=ALU.is_ge,
                fill=0.0, base=0, pattern=[[1, 128]], channel_multiplier=-1)

            # denominator column (65th) is constant 1; padding handled in exp
            nc.gpsimd.memset(v_r[:, :, :, 64], 1.0)

            with tc.tile_pool(name="psP", bufs=2, space="PSUM") as psP, \
                 tc.tile_pool(name="psS", bufs=2, space="PSUM") as psS, \
                 tc.tile_pool(name="psAv", bufs=1, space="PSUM") as psAv, \
                 tc.tile_pool(name="attp", bufs=4) as attp, \
                 tc.tile_pool(name="outp", bufs=3) as outp, \
                 tc.tile_pool(name="wkp", bufs=6) as wkp, \
                 tc.tile_pool(name="bcp", bufs=3) as bcp:

                def qk_group(scn, ob, which):
                    q0 = scn * 512
                    wsb, bias_sb, dst = ((wq_sb, bq_sb, qT_sb),
                                         (wk_sb, bk_sb, kT_sb))[which]
                    ps = psP.tile([128, 512], F32, tag="ps_proj")
                    for eb in range(EB):
                        nc.tensor.matmul(
                            ps[:],
                            wsb[:, eb * OC + ob * 128:
                                eb * OC + (ob + 1) * 128],
                            x_sb[:, eb * S + q0:eb * S + q0 + 512],
                            start=(eb == 0), stop=(eb == EB - 1))
                    if which == 0:
                        # qT pre-scaled by 0.125*128/ln2: scores arrive in
                        # Schraudolph/exp-ready units
                        nc.vector.tensor_scalar(
                            dst[:, ob * S + q0:ob * S + q0 + 512],
                            ps[:], bias_sb[:, ob:ob + 1],
                            0.125 * SCH_A, ALU.add, ALU.mult)
                    else:
                        nc.vector.tensor_scalar_add(
                            dst[:, ob * S + q0:ob * S + q0 + 512],
                            ps[:], bias_sb[:, ob:ob + 1])

                def v_group(sb):
                    ps = psP.tile([128, 512], F32, tag="ps_proj")
                    for eb in range(EB):
                        nc.tensor.matmul(
                            ps[:],
                            x_sb[:, eb * S + sb * 128:eb * S + (sb + 1) * 128],
                            wv_sb[:, eb * OC:(eb + 1) * OC],
                            start=(eb == 0), stop=(eb == EB - 1))
                    nc.vector.tensor_add(
                        v_r[:, sb, :, 0:64],
                        ps[:].rearrange("p (h c) -> p h c", h=HPC),
                        bv_tile[:].rearrange("p (h c) -> p h c", h=HPC))

                def proj_thunks(scn):
                    t = []
                    for ob in range(4):
                        t.append(lambda scn=scn, ob=ob: qk_group(scn, ob, 0))
                        t.append(lambda scn=scn, ob=ob: qk_group(scn, ob, 1))
                    for sb in range(4 * scn, 4 * scn + 4):
                        t.append(lambda sb=sb: v_group(sb))
                    return t

                # ---- minimal upfront work: only what attn(0, hp0) needs ----
                qk_group(0, 0, 0)
                qk_group(0, 0, 1)
                # chunk-0 leftovers are pumped into the attention slots in an
                # order that stays ahead of each consumer (v(kb) before its
                # AV, qk(ob) before head-pair ob)
                chunk0_rest = [
                    lambda: v_group(0),
                    lambda: qk_group(0, 1, 0),
                    lambda: v_group(1),
                    lambda: qk_group(0, 1, 1),
                    lambda: v_group(2),
                    lambda: v_group(3),
                    lambda: qk_group(0, 2, 0),
                    lambda: qk_group(0, 2, 1),
                    lambda: qk_group(0, 3, 0),
                    lambda: qk_group(0, 3, 1),
                ]

                # ---- attention scn with proj(scn+1) interleaved ----
                for scn in range(NCH):
                    q0 = scn * 512
                    nkb = 4 * scn + 4
                    work = list(chunk0_rest) if scn == 0 else []
                    chunk0_rest = []
                    if scn + 1 < NCH:
                        work += proj_thunks(scn + 1)
                    slots_left = 4 * nkb

                    for hp in range(4):
                        heads = (2 * hp, 2 * hp + 1)
                        avs2 = psAv.tile([65, 1024], F32, tag="ps_av")
                        prev = None
                        for kb in range(nkb):
                            lstart = max(0, kb * 128 - q0)
                            w = 512 - lstart
                            ps2 = psS.tile([128, 1024], F32, tag="ps_s")
                            for i, h in enumerate(heads):
                                ob = h // 2
                                po = (h % 2) * 64
                                nc.tensor.matmul(
                                    ps2[:, i * 512:i * 512 + w],
                                    kT_sb[po:po + 64,
                                          ob * S + kb * 128:
                                          ob * S + (kb + 1) * 128],
                                    qT_sb[po:po + 64,
                                          ob * S + q0 + lstart:
                                          ob * S + q0 + 512],
                                    start=True, stop=True,
                                    tile_position=(po, 0))
                            att2 = attp.tile([128, 1024], BF16, tag="att")
                            if w == 512:
                                pv, av = ps2[:], att2[:]
                            else:
                                pv = ps2[:].rearrange(
                                    "p (t c) -> p t c", t=2)[:, :, 0:w]
                                av = att2[:].rearrange(
                                    "p (t c) -> p t c", t=2)[:, :, 0:w]
                            if scn == 3 and kb % 2 == 1:
                                avi = att2[:].bitcast(mybir.dt.int16)
                                if w != 512:
                                    avi = avi.rearrange(
                                        "p (t c) -> p t c", t=2)[:, :, 0:w]
                                nc.vector.tensor_scalar(
                                    avi, pv, sch_bias[:, kb:kb + 1], 0.0,
                                    ALU.add, ALU.max)
                            else:
                                nc.scalar.activation(
                                    av, pv, AF.Exp, scale=1.0 / SCH_A,
                                    bias=padlog[:, kb:kb + 1])
                            if kb >= 4 * scn:
                                for i in range(2):
                                    nc.vector.tensor_mul(
                                        att2[:, i * 512:i * 512 + 128],
                                        att2[:, i * 512:i * 512 + 128],
                                        tri[:])
                            # interleave some projection work for scn+1
                            if work:
                                n = -(-len(work) // slots_left)
                                for th in work[:n]:
                                    th()
                                del work[:n]
                            slots_left -= 1
                            if prev is not None:
                                p_att, p_lstart, p_w, p_kb = prev
                                for i, h in enumerate(heads):
                                    nc.tensor.matmul(
                                        avs2[:, i * 512 + p_lstart:
                                             i * 512 + 512],
                                        v_r[:, p_kb, h, :],
                                        p_att[:, i * 512:i * 512 + p_w],
                                        start=(p_kb == 0),
                                        stop=(p_kb == nkb - 1))
                            prev = (att2, lstart, w, kb)
                        p_att, p_lstart, p_w, p_kb = prev
                        for i, h in enumerate(heads):
                            nc.tensor.matmul(
                                avs2[:, i * 512 + p_lstart:i * 512 + 512],
                                v_r[:, p_kb, h, :],
                                p_att[:, i * 512:i * 512 + p_w],
                                start=(p_kb == 0), stop=(p_kb == nkb - 1))

                        # ---- normalize + store ----
                        av_sb2 = outp.tile([65, 1024], F32, tag="av_sb")
                        nc.vector.tensor_copy(av_sb2[:], avs2[:])
                        for i, h in enumerate(heads):
                            av_sb = av_sb2[:, i * 512:(i + 1) * 512]
                            rden = wkp.tile([1, 512], F32, tag="rt",
                                            name="rden")
                            nc.vector.tensor_scalar_add(
                                rden[:], av_sb2[64:65, i * 512:(i + 1) * 512],
                                1e-30)
                            rrec = wkp.tile([1, 512], F32, tag="rt",
                                            name="rrec")
                            nc.vector.reciprocal_approx_fast(rrec[:], rden[:])
                            rpad = wkp.tile([1, 512], F32, tag="rt",
                                            name="rpad")
                            nc.vector.tensor_mul(rpad[:], rrec[:],
                                                 pad_row[:, q0:q0 + 512])
                            bc = bcp.tile([64, 512], F32, tag="bc")
                            nc.gpsimd.partition_broadcast(bc[:], rpad[:])
                            nc.vector.tensor_mul(
                                av_sb2[0:64, i * 512:(i + 1) * 512],
                                av_sb2[0:64, i * 512:(i + 1) * 512], bc[:])
                            nc.sync.dma_start(
                                outT[h * 64:(h + 1) * 64, q0:q0 + 512],
                                av_sb2[0:64, i * 512:(i + 1) * 512])

                    # drain any leftover projection work for scn+1
                    for th in work:
                        th()
    nc.compile()
    return nc


def get_nc():
    if "nc" not in _cache:
        _cache["nc"] = _build_nc()
    return _cache["nc"]


def make_in_maps(input_x, pad_mask, Wq, bq, Wk, bk, Wv, bv):
    bf16 = ml_dtypes.bfloat16
    input_x = np.asarray(input_x, dtype=np.float32)
    pad_f = np.asarray(pad_mask).astype(np.float32)
    Wq = np.asarray(Wq, dtype=np.float32)
    Wk = np.asarray(Wk, dtype=np.float32)
    Wv = np.asarray(Wv, dtype=np.float32)
    bq = np.asarray(bq, dtype=np.float32)
    bk = np.asarray(bk, dtype=np.float32)
    bv = np.asarray(bv, dtype=np.float32)

    xTs = [np.ascontiguousarray(input_x[b].T).astype(bf16) for b in range(B)]
    wslices = {}
    for g in range(2):
        sl = slice(g * OC, (g + 1) * OC)
        wslices[g] = (np.ascontiguousarray(Wq[sl].T).astype(bf16),
                      np.ascontiguousarray(Wk[sl].T).astype(bf16),
                      np.ascontiguousarray(Wv[sl].T).astype(bf16),
                      np.ascontiguousarray(bq[sl]),
                      np.ascontiguousarray(bk[sl]),
                      np.ascontiguousarray(bv[sl]))
    in_maps = []
    for c in range(NCORES):
        b, g = c // 2, c % 2
        wq_t, wk_t, wv_t, bq_s, bk_s, bv_s = wslices[g]
        in_maps.append({
            "xT": xTs[b], "wqT": wq_t, "wkT": wk_t, "wvT": wv_t,
            "bq": bq_s, "bk": bk_s, "bv": bv_s,
            "pad": np.ascontiguousarray(pad_f[b]),
        })
    return in_maps


def assemble(results):
    out = np.empty((B, S, E), dtype=np.float32)
    for c in range(NCORES):
        b, g = c // 2, c % 2
        out[b, :, g * OC:(g + 1) * OC] = results[c]["outT"].T
    return out


_last_result = None


def kernel(input_x, pad_mask, Wq, bq, Wk, bk, Wv, bv):
    global _last_result
    from concourse.bass_utils import run_bass_kernel_spmd
    nc = get_nc()
    in_maps = make_in_maps(input_x, pad_mask, Wq, bq, Wk, bk, Wv, bv)
    res = run_bass_kernel_spmd(nc, in_maps, core_ids=list(range(NCORES)))
    _last_result = res
    if res.exec_time_ns is not None:
        print(f"HW exec time: {res.exec_time_ns} ns")
    return assemble(res.results)


# revision 17
# speedup vs baseline: 1.0406x; 1.0406x over previous
"""Multi-head causal+padded attention on 8 Trainium2 NeuronCores.

Sharding: core c handles batch b = c//2 and head-group g = c%2 (8 of 16 heads).
Each core computes its q/k/v projections (512 output dims) and attention for
its 8 heads over the full 2048-seq, producing out^T [512, 2048]; the host
transposes/concats into the full [4, 2048, 1024] output.

Device algorithm (per core), v2:
  All matmul operands bf16 (fp32 PSUM accumulation). xT resident in SBUF;
  qT/kT = W^T-slices @ xT ([outdim, seq] layout); v natural [seq, outdim],
  augmented with a 65th all-ones column whose att-weighted sum is the softmax
  denominator. Key-side pad masking is folded into the exp bias (-87 for
  padded keys). Scores for a head pair are computed as two row-tiled matmuls
  (partitions 0-63 / 64-127 of the contraction) into adjacent PSUM banks and
  exp'd by a single paired ACT instruction. AV matmuls run one k-block behind
  QK/exp (software pipeline). Projections for seq-chunk scn+1 are interleaved
  into the attention of chunk scn as micro-ops to keep the PE busy while the
  ACT engine drains exp work. Normalization: copy PSUM->SBUF, fast-approx
  reciprocal of the denominator row, pad(q) fold, gpsimd partition broadcast,
  one elementwise multiply.
"""
import os
import sys

sys.path.insert(0, "/opt/trn_rl_repo")

import numpy as np
import ml_dtypes

S = 2048
E = 1024
D = 64
H = 16          # total heads
HPC = 8         # heads per core
OC = HPC * D    # 512 output dims per core
EB = E // 128   # 8 contraction blocks
NSB = S // 128  # 16 seq blocks
NCH = S // 512  # 4 q-chunks
B = 4
NCORES = 8

_cache = {}


def _build_nc():
    from concourse import bacc
    import concourse.tile as tile
    import concourse.mybir as mybir

    F32 = mybir.dt.float32
    BF16 = mybir.dt.bfloat16
    AF = mybir.ActivationFunctionType
    ALU = mybir.AluOpType

    nc = bacc.Bacc("TRN2", target_bir_lowering=False, debug=False,
                   num_devices=NCORES)
    xT = nc.dram_tensor("xT", [E, S], BF16, kind="ExternalInput").ap()
    wqT = nc.dram_tensor("wqT", [E, OC], BF16, kind="ExternalInput").ap()
    wkT = nc.dram_tensor("wkT", [E, OC], BF16, kind="ExternalInput").ap()
    wvT = nc.dram_tensor("wvT", [E, OC], BF16, kind="ExternalInput").ap()
    bq = nc.dram_tensor("bq", [OC], F32, kind="ExternalInput").ap()
    bk = nc.dram_tensor("bk", [OC], F32, kind="ExternalInput").ap()
    bv = nc.dram_tensor("bv", [OC], F32, kind="ExternalInput").ap()
    pad = nc.dram_tensor("pad", [S], F32, kind="ExternalInput").ap()
    outT = nc.dram_tensor("outT", [OC, S], F32, kind="ExternalOutput").ap()

    with tile.TileContext(nc) as tc:
        with tc.tile_pool(name="const", bufs=1) as cpool, \
             tc.tile_pool(name="big", bufs=1) as bigpool:

            # ---------------- big SBUF residents ----------------
            x_sb = bigpool.tile([128, EB * S], BF16, tag="x_sb")
            wq_sb = bigpool.tile([128, EB * OC], BF16, tag="wq_sb")
            wk_sb = bigpool.tile([128, EB * OC], BF16, tag="wk_sb")
            wv_sb = bigpool.tile([128, EB * OC], BF16, tag="wv_sb")
            qT_sb = bigpool.tile([128, 4 * S], BF16, tag="qT")
            kT_sb = bigpool.tile([128, 4 * S], BF16, tag="kT")
            v_aug = bigpool.tile([128, NSB * HPC * 65], BF16, tag="v_aug")
            v_r = v_aug[:].rearrange("p (b h c) -> p b h c", b=NSB, h=HPC)

            # DMA order = critical path of attn(0, hp0): all x blocks, the
            # ob0 column-slices of wq/wk, wv, then the weight remainders.
            for eb in range(EB):
                nc.sync.dma_start(x_sb[:, eb * S:(eb + 1) * S],
                                  xT[eb * 128:(eb + 1) * 128, :])
            wq_v = wq_sb[:].rearrange("p (e c) -> p e c", e=EB)
            wk_v = wk_sb[:].rearrange("p (e c) -> p e c", e=EB)
            wqT_v = wqT.rearrange("(e p) c -> p e c", p=128)
            wkT_v = wkT.rearrange("(e p) c -> p e c", p=128)
            nc.sync.dma_start(wq_v[:, :, 0:128], wqT_v[:, :, 0:128])
            nc.sync.dma_start(wk_v[:, :, 0:128], wkT_v[:, :, 0:128])
            nc.sync.dma_start(
                wv_sb[:].rearrange("p (e c) -> p e c", e=EB),
                wvT.rearrange("(e p) c -> p e c", p=128))
            nc.sync.dma_start(wq_v[:, :, 128:OC], wqT_v[:, :, 128:OC])
            nc.sync.dma_start(wk_v[:, :, 128:OC], wkT_v[:, :, 128:OC])

            # ---------------- constants ----------------
            pad_sb = cpool.tile([128, NSB], F32, tag="pad_f")
            nc.sync.dma_start(pad_sb[:], pad.rearrange("(b p) -> p b", p=128))
            pad_row = cpool.tile([1, S], F32, tag="padr_f")
            nc.sync.dma_start(pad_row[:], pad.rearrange("(a s) -> a s", a=1))
            # exp bias: 0 where pad=1, -87 where pad=0 (folds key padding)
            padlog = cpool.tile([128, NSB], F32, tag="padlog")
            nc.vector.tensor_scalar(padlog[:], pad_sb[:], 87.0, -87.0,
                                    ALU.mult, ALU.add)

            bq_sb = cpool.tile([128, 4], F32, tag="bq")
            nc.sync.dma_start(bq_sb[:], bq.rearrange("(b p) -> p b", p=128))
            bk_sb = cpool.tile([128, 4], F32, tag="bk")
            nc.sync.dma_start(bk_sb[:], bk.rearrange("(b p) -> p b", p=128))
            bv_row = cpool.tile([1, OC], F32, tag="bv_row")
            nc.sync.dma_start(bv_row[:], bv.rearrange("(a c) -> a c", a=1))
            bv_tile = cpool.tile([128, OC], F32, tag="bv_tile")
            nc.gpsimd.partition_broadcast(bv_tile[:], bv_row[:])

            # tri[k, q] = 1 where k <= q else 0 (local 128x128 diagonal block)
            tri = cpool.tile([128, 128], BF16, tag="tri")
            nc.gpsimd.memset(tri[:], 1.0)
            nc.gpsimd.affine_select(
                out=tri[:], in_=tri[:], compare_op=ALU.is_ge,
                fill=0.0, base=0, pattern=[[1, 128]], channel_multiplier=-1)

            # denominator column (65th) is constant 1; padding handled in exp
            nc.gpsimd.memset(v_r[:, :, :, 64], 1.0)

            with tc.tile_pool(name="psP", bufs=2, space="PSUM") as psP, \
                 tc.tile_pool(name="psS", bufs=2, space="PSUM") as psS, \
                 tc.tile_pool(name="psAv", bufs=1, space="PSUM") as psAv, \
                 tc.tile_pool(name="attp", bufs=7) as attp, \
                 tc.tile_pool(name="outp", bufs=4) as outp, \
                 tc.tile_pool(name="wkp", bufs=8) as wkp, \
                 tc.tile_pool(name="bcp", bufs=4) as bcp:

                def qk_group(scn, ob, which):
                    q0 = scn * 512
                    wsb, bias_sb, dst = ((wq_sb, bq_sb, qT_sb),
                                         (wk_sb, bk_sb, kT_sb))[which]
                    ps = psP.tile([128, 512], F32, tag="ps_proj")
                    for eb in range(EB):
                        nc.tensor.matmul(
                            ps[:],
                            wsb[:, eb * OC + ob * 128:
                                eb * OC + (ob + 1) * 128],
                            x_sb[:, eb * S + q0:eb * S + q0 + 512],
                            start=(eb == 0), stop=(eb == EB - 1))
                    nc.vector.tensor_scalar_add(
                        dst[:, ob * S + q0:ob * S + q0 + 512],
                        ps[:], bias_sb[:, ob:ob + 1])

                def v_group(sb):
                    ps = psP.tile([128, 512], F32, tag="ps_proj")
                    for eb in range(EB):
                        nc.tensor.matmul(
                            ps[:],
                            x_sb[:, eb * S + sb * 128:eb * S + (sb + 1) * 128],
                            wv_sb[:, eb * OC:(eb + 1) * OC],
                            start=(eb == 0), stop=(eb == EB - 1))
                    nc.vector.tensor_add(
                        v_r[:, sb, :, 0:64],
                        ps[:].rearrange("p (h c) -> p h c", h=HPC),
                        bv_tile[:].rearrange("p (h c) -> p h c", h=HPC))

                def proj_thunks(scn):
                    t = []
                    for ob in range(4):
                        t.append(lambda scn=scn, ob=ob: qk_group(scn, ob, 0))
                        t.append(lambda scn=scn, ob=ob: qk_group(scn, ob, 1))
                    for sb in range(4 * scn, 4 * scn + 4):
                        t.append(lambda sb=sb: v_group(sb))
                    return t

                # ---- minimal upfront work: only what attn(0, hp0) needs ----
                qk_group(0, 0, 0)
                qk_group(0, 0, 1)
                # chunk-0 leftovers are pumped into the attention slots in an
                # order that stays ahead of each consumer (v(kb) before its
                # AV, qk(ob) before head-pair ob)
                chunk0_rest = [
                    lambda: v_group(0),
                    lambda: qk_group(0, 1, 0),
                    lambda: v_group(1),
                    lambda: qk_group(0, 1, 1),
                    lambda: v_group(2),
                    lambda: v_group(3),
                    lambda: qk_group(0, 2, 0),
                    lambda: qk_group(0, 2, 1),
                    lambda: qk_group(0, 3, 0),
                    lambda: qk_group(0, 3, 1),
                ]

                # ---- attention scn with proj(scn+1) interleaved ----
                for scn in range(NCH):
                    q0 = scn * 512
                    nkb = 4 * scn + 4
                    work = list(chunk0_rest) if scn == 0 else []
                    chunk0_rest = []
                    if scn + 1 < NCH:
                        work += proj_thunks(scn + 1)
                    slots_left = 4 * nkb

                    for hp in range(4):
                        heads = (2 * hp, 2 * hp + 1)
                        avs2 = psAv.tile([65, 1024], F32, tag="ps_av")
                        prev = None
                        for kb in range(nkb):
                            lstart = max(0, kb * 128 - q0)
                            w = 512 - lstart
                            ps2 = psS.tile([128, 1024], F32, tag="ps_s")
                            for i, h in enumerate(heads):
                                ob = h // 2
                                po = (h % 2) * 64
                                nc.tensor.matmul(
                                    ps2[:, i * 512:i * 512 + w],
                                    kT_sb[po:po + 64,
                                          ob * S + kb * 128:
                                          ob * S + (kb + 1) * 128],
                                    qT_sb[po:po + 64,
                                          ob * S + q0 + lstart:
                                          ob * S + q0 + 512],
                                    start=True, stop=True,
                                    tile_position=(po, 0))
                            att2 = attp.tile([128, 1024], BF16, tag="att")
                            if w == 512:
                                nc.scalar.activation(
                                    att2[:], ps2[:], AF.Exp, scale=0.125,
                                    bias=padlog[:, kb:kb + 1])
                            else:
                                pv = ps2[:].rearrange(
                                    "p (t c) -> p t c", t=2)[:, :, 0:w]
                                av = att2[:].rearrange(
                                    "p (t c) -> p t c", t=2)[:, :, 0:w]
                                nc.scalar.activation(
                                    av, pv, AF.Exp, scale=0.125,
                                    bias=padlog[:, kb:kb + 1])
                            if kb >= 4 * scn:
                                for i in range(2):
                                    nc.vector.tensor_mul(
                                        att2[:, i * 512:i * 512 + 128],
                                        att2[:, i * 512:i * 512 + 128],
                                        tri[:])
                            # interleave some projection work for scn+1
                            if work:
                                n = -(-len(work) // slots_left)
                                for th in work[:n]:
                                    th()
                                del work[:n]
                            slots_left -= 1
                            if prev is not None:
                                p_att, p_lstart, p_w, p_kb = prev
                                for i, h in enumerate(heads):
                                    nc.tensor.matmul(
                                        avs2[:, i * 512 + p_lstart:
                                             i * 512 + 512],
                                        v_r[:, p_kb, h, :],
                                        p_att[:, i * 512:i * 512 + p_w],
                                        start=(p_kb == 0),
                                        stop=(p_kb == nkb - 1))
                            prev = (att2, lstart, w, kb)
                        p_att, p_lstart, p_w, p_kb = prev
                        for i, h in enumerate(heads):
                            nc.tensor.matmul(
                                avs2[:, i * 512 + p_lstart:i * 512 + 512],
                                v_r[:, p_kb, h, :],
                                p_att[:, i * 512:i * 512 + p_w],
                                start=(p_kb == 0), stop=(p_kb == nkb - 1))

                        # ---- normalize + store ----
                        av_sb2 = outp.tile([65, 1024], F32, tag="av_sb")
                        nc.vector.tensor_copy(av_sb2[:], avs2[:])
                        for i, h in enumerate(heads):
                            av_sb = av_sb2[:, i * 512:(i + 1) * 512]
                            rden = wkp.tile([1, 512], F32, tag="rt",
                                            name="rden")
                            nc.vector.tensor_scalar_add(
                                rden[:], av_sb2[64:65, i * 512:(i + 1) * 512],
                                1e-30)
                            rrec = wkp.tile([1, 512], F32, tag="rt",
                                            name="rrec")
                            nc.vector.reciprocal_approx_fast(rrec[:], rden[:])
                            rpad = wkp.tile([1, 512], F32, tag="rt",
                                            name="rpad")
                            nc.vector.tensor_mul(rpad[:], rrec[:],
                                                 pad_row[:, q0:q0 + 512])
                            bc = bcp.tile([64, 512], F32, tag="bc")
                            nc.gpsimd.partition_broadcast(bc[:], rpad[:])
                            nc.vector.tensor_mul(
                                av_sb2[0:64, i * 512:(i + 1) * 512],
                                av_sb2[0:64, i * 512:(i + 1) * 512], bc[:])
                            nc.sync.dma_start(
                                outT[h * 64:(h + 1) * 64, q0:q0 + 512],
                                av_sb2[0:64, i * 512:(i + 1) * 512])

                    # drain any leftover projection work for scn+1
                    for th in work:
                        th()
    nc.compile()
    return nc


def get_nc():
    if "nc" not in _cache:
        _cache["nc"] = _build_nc()
    return _cache["nc"]


def make_in_maps(input_x, pad_mask, Wq, bq, Wk, bk, Wv, bv):
    bf16 = ml_dtypes.bfloat16
    input_x = np.asarray(input_x, dtype=np.float32)
    pad_f = np.asarray(pad_mask).astype(np.float32)
    Wq = np.asarray(Wq, dtype=np.float32)
    Wk = np.asarray(Wk, dtype=np.float32)
    Wv = np.asarray(Wv, dtype=np.float32)
    bq = np.asarray(bq, dtype=np.float32)
    bk = np.asarray(bk, dtype=np.float32)
    bv = np.asarray(bv, dtype=np.float32)

    xTs = [np.ascontiguousarray(input_x[b].T).astype(bf16) for b in range(B)]
    wslices = {}
    for g in range(2):
        sl = slice(g * OC, (g + 1) * OC)
        wslices[g] = (np.ascontiguousarray(Wq[sl].T).astype(bf16),
                      np.ascontiguousarray(Wk[sl].T).astype(bf16),
                      np.ascontiguousarray(Wv[sl].T).astype(bf16),
                      np.ascontiguousarray(bq[sl]),
                      np.ascontiguousarray(bk[sl]),
                      np.ascontiguousarray(bv[sl]))
    in_maps = []
    for c in range(NCORES):
        b, g = c // 2, c % 2
        wq_t, wk_t, wv_t, bq_s, bk_s, bv_s = wslices[g]
        in_maps.append({
            "xT": xTs[b], "wqT": wq_t, "wkT": wk_t, "wvT": wv_t,
            "bq": bq_s, "bk": bk_s, "bv": bv_s,
            "pad": np.ascontiguousarray(pad_f[b]),
        })
    return in_maps


def assemble(results):
    out = np.empty((B, S, E), dtype=np.float32)
    for c in range(NCORES):
        b, g = c // 2, c % 2
        out[b, :, g * OC:(g + 1) * OC] = results[c]["outT"].T
    return out


_last_result = None


def kernel(input_x, pad_mask, Wq, bq, Wk, bk, Wv, bv):
    global _last_result
    from concourse.bass_utils import run_bass_kernel_spmd
    nc = get_nc()
    in_maps = make_in_maps(input_x, pad_mask, Wq, bq, Wk, bk, Wv, bv)
    res = run_bass_kernel_spmd(nc, in_maps, core_ids=list(range(NCORES)))
    _last_result = res
    if res.exec_time_ns is not None:
        print(f"HW exec time: {res.exec_time_ns} ns")
    return assemble(res.results)
